# revision 2
# baseline (speedup 1.0000x reference)
import numpy as np
import ml_dtypes
import concourse.bass as bass
import concourse.bacc as bacc
import concourse.tile as tile
import concourse.mybir as mybir
from concourse import bass_utils
from contextlib import ExitStack

B = 4
QL = 1024
HIST = 1024
KVL = 2048
H = 4096
NH = 32
D = 128
T = 4096
NCORES = 8
HPC = NH // NCORES          # 4 heads per core
ROPE_BASE = 10000.0
INV_NORM = 1.0 / float(np.sqrt(D))
NEG = -1.0e30

FP = mybir.dt.float32
BF = mybir.dt.bfloat16
F16 = mybir.dt.float16
AX = mybir.AluOpType
AF = mybir.ActivationFunctionType
BF_NP = ml_dtypes.bfloat16

LAG = 3   # attention PV software-pipeline depth (groups)


def _build():
    nc = bacc.Bacc("TRN2", num_devices=NCORES)
    xT = nc.dram_tensor("xT", [H, T], BF, kind="ExternalInput")
    wqk2 = nc.dram_tensor("wqk2", [2, H, HPC * D], BF, kind="ExternalInput")
    w_v = nc.dram_tensor("w_v", [H, HPC * D], BF, kind="ExternalInput")
    wd = nc.dram_tensor("wd", [HPC * D, H], BF, kind="ExternalInput")
    kTh = nc.dram_tensor("kTh", [B, HPC, D, HIST], BF, kind="ExternalInput")
    vhp = nc.dram_tensor("vhp", [B, HPC, 128, HIST], BF, kind="ExternalInput")
    cosT = nc.dram_tensor("cosT", [D, T], FP, kind="ExternalInput")
    sinT = nc.dram_tensor("sinT", [D, T], FP, kind="ExternalInput")
    masksAB = nc.dram_tensor("masksAB", [2, D, 1024], FP, kind="ExternalInput")
    outT = nc.dram_tensor("outT", [H, T], F16, kind="ExternalOutput")

    with tile.TileContext(nc) as tc, ExitStack() as top:
        # ---- constants ----
        cpool = top.enter_context(tc.tile_pool(name="const", bufs=1))
        ones_f = cpool.tile([128, 128], FP)
        nc.vector.memset(ones_f[:, :], 1.0)
        ones128 = cpool.tile([128, 128], BF)
        nc.scalar.copy(ones128[:, :], ones_f[:, :])

        # ---- resident weights / masks ----
        wvp = top.enter_context(tc.tile_pool(name="wvp", bufs=1))
        wdp = top.enter_context(tc.tile_pool(name="wdp", bufs=1))
        mkp = top.enter_context(tc.tile_pool(name="mkp", bufs=1))
        wv_sb = [wvp.tile([128, HPC * D], BF, name=f"wv{k}") for k in range(32)]
        wd_sb = [wdp.tile([128, H], BF, name=f"wd{k}") for k in range(HPC)]
        mask_t = [mkp.tile([128, 1024], FP, name=f"mask{d}") for d in range(2)]

        # ---- rotating pools ----
        xp = top.enter_context(tc.tile_pool(name="xp", bufs=1))
        wqp = top.enter_context(tc.tile_pool(name="wqp", bufs=1))
        qkp = top.enter_context(tc.tile_pool(name="qkp", bufs=1))
        vsp = top.enter_context(tc.tile_pool(name="vsp", bufs=1))
        atp = top.enter_context(tc.tile_pool(name="atp", bufs=1))
        csp = top.enter_context(tc.tile_pool(name="csp", bufs=1))
        khp = top.enter_context(tc.tile_pool(name="khp", bufs=1))
        vhp_p = top.enter_context(tc.tile_pool(name="vhp", bufs=1))
        rotp = top.enter_context(tc.tile_pool(name="rotp", bufs=1))
        pp = top.enter_context(tc.tile_pool(name="pp", bufs=1))
        pap = top.enter_context(tc.tile_pool(name="pap", bufs=1))
        sxp = top.enter_context(tc.tile_pool(name="sxp", bufs=1))
        recp = top.enter_context(tc.tile_pool(name="recp", bufs=1))
        op_ = top.enter_context(tc.tile_pool(name="op", bufs=1))

        cs_t = [None, None]
        at_prev = None
        c0_prev = 0

        def emit_dense(ms, at_src, c0_dst, psC):
            # dense out-proj m-blocks using at_src ([HPC][2] of [128,512] bf16)
            for i, m in enumerate(ms):
                for qc2 in range(2):
                    pso = psC.tile([128, 512], FP, name="pso", bufs=2)
                    for k4 in range(HPC):
                        nc.tensor.matmul(
                            pso[:, :], wd_sb[k4][:, m * 128:(m + 1) * 128],
                            at_src[k4][qc2][:, :],
                            start=(k4 == 0), stop=(k4 == HPC - 1))
                    o = op_.tile([128, 512], F16, name="o", bufs=4)
                    if (i + qc2) % 2 == 0:
                        nc.scalar.copy(o[:, :], pso[:, :])
                    else:
                        nc.vector.tensor_copy(o[:, :], pso[:, :])
                    nc.sync.dma_start(
                        out=outT[m * 128:(m + 1) * 128,
                                 c0_dst + qc2 * 512:c0_dst + qc2 * 512 + 512],
                        in_=o[:, :])

        for b in range(B):
            c0 = b * QL
            qk_t = [None] * 8
            v_sb = []
            kh_t = [None] * HPC
            vh_t = [None] * HPC

            # ============ stage A1: q^T,k^T = w^T x with RoPE ============
            with ExitStack() as actx:
                psA = actx.enter_context(
                    tc.tile_pool(name=f"psA{b}", bufs=1, space="PSUM"))
                for sweep in range(2):
                    psa = [psA.tile([128, 1024], FP, name=f"psa{m}", bufs=1)
                           for m in range(4)]
                    for k in range(32):
                        xt = xp.tile([128, 1024], BF, name="x", bufs=6)
                        nc.sync.dma_start(
                            out=xt[:, :], in_=xT[k * 128:(k + 1) * 128, c0:c0 + QL])
                        wq = wqp.tile([128, 512], BF, name="wq", bufs=6)
                        nc.sync.dma_start(
                            out=wq[:, :], in_=wqk2[sweep, k * 128:(k + 1) * 128, :])
                        for m4 in range(4):
                            for ns in range(2):
                                nc.tensor.matmul(
                                    psa[m4][:, ns * 512:(ns + 1) * 512],
                                    wq[:, m4 * 128:(m4 + 1) * 128],
                                    xt[:, ns * 512:(ns + 1) * 512],
                                    start=(k == 0), stop=(k == 31))
                    if sweep == 0:
                        ct = csp.tile([128, 1024], FP, name="cos")
                        nc.sync.dma_start(out=ct[:, :], in_=cosT[:, c0:c0 + QL])
                        st = csp.tile([128, 1024], FP, name="sin")
                        nc.sync.dma_start(out=st[:, :], in_=sinT[:, c0:c0 + QL])
                        cs_t[0], cs_t[1] = ct, st
                        if b == 0:
                            for k in range(32):
                                nc.sync.dma_start(
                                    out=wv_sb[k][:, :],
                                    in_=w_v[k * 128:(k + 1) * 128, :])
                    else:
                        if b == 0:
                            for k in range(HPC):
                                nc.sync.dma_start(
                                    out=wd_sb[k][:, :],
                                    in_=wd[k * 128:(k + 1) * 128, :])
                            for d in range(2):
                                nc.sync.dma_start(
                                    out=mask_t[d][:, :], in_=masksAB[d, :, :])
                    # rope drain: qk = psa*cos + rotate_half(psa)*sin
                    for m4 in range(4):
                        rot = rotp.tile([128, 1024], FP, name="rot", bufs=2)
                        nc.scalar.mul(rot[0:64, :], psa[m4][64:128, :], -1.0)
                        nc.scalar.copy(rot[64:128, :], psa[m4][0:64, :])
                        qk = qkp.tile([128, 1024], BF, name=f"qk{sweep * 4 + m4}")
                        nc.vector.tensor_tensor(
                            out=qk[:, :], in0=psa[m4][:, :], in1=cs_t[0][:, :],
                            op=AX.mult)
                        nc.vector.tensor_tensor(
                            out=rot[:, :], in0=rot[:, :], in1=cs_t[1][:, :],
                            op=AX.mult)
                        nc.vector.tensor_tensor(
                            out=qk[:, :], in0=qk[:, :], in1=rot[:, :], op=AX.add)
                        qk_t[sweep * 4 + m4] = qk

            # ============ stage A2: new V (token-major) ============
            with ExitStack() as a2ctx:
                psV = a2ctx.enter_context(
                    tc.tile_pool(name=f"psV{b}", bufs=1, space="PSUM"))
                psv = [psV.tile([128, 512], FP, name=f"psv{t}", bufs=1)
                       for t in range(8)]
                for k in range(32):
                    xt = xp.tile([128, 1024], BF, name="x", bufs=6)
                    nc.sync.dma_start(
                        out=xt[:, :], in_=xT[k * 128:(k + 1) * 128, c0:c0 + QL])
                    for t in range(8):
                        nc.tensor.matmul(
                            psv[t][:, :], xt[:, t * 128:(t + 1) * 128],
                            wv_sb[k][:, :], start=(k == 0), stop=(k == 31))
                # prefetch this b's history K/V
                for h2 in range(HPC):
                    kh = khp.tile([128, HIST], BF, name=f"kh{h2}")
                    nc.sync.dma_start(out=kh[:, :], in_=kTh[b, h2, :, :])
                    kh_t[h2] = kh
                    vh = vhp_p.tile([128, HIST], BF, name=f"vh{h2}")
                    nc.sync.dma_start(out=vh[:, :], in_=vhp[b, h2, :, :])
                    vh_t[h2] = vh
                for t in range(8):
                    vt = vsp.tile([128, 512], BF, name=f"v{t}")
                    if t % 2 == 0:
                        nc.scalar.copy(vt[:, :], psv[t][:, :])
                    else:
                        nc.vector.tensor_copy(vt[:, :], psv[t][:, :])
                    v_sb.append(vt)

            # ============ stage B(b) attention ⊗ C(b-1) dense ============
            at_cur = [[None, None] for _ in range(HPC)]
            with ExitStack() as bctx:
                psSG = bctx.enter_context(
                    tc.tile_pool(name=f"psSG{b}", bufs=1, space="PSUM"))
                psAT = bctx.enter_context(
                    tc.tile_pool(name=f"psAT{b}", bufs=1, space="PSUM"))
                psDN = bctx.enter_context(
                    tc.tile_pool(name=f"psDN{b}", bufs=1, space="PSUM"))
                psC = bctx.enter_context(
                    tc.tile_pool(name=f"psC{b}", bufs=1, space="PSUM"))

                for h in range(HPC):
                    def k_src(ti, h=h):
                        if ti < 8:
                            return kh_t[h][:, ti * 128:(ti + 1) * 128]
                        return qk_t[4 + h][:, (ti - 8) * 128:(ti - 7) * 128]

                    def v_src(ti, h=h):
                        if ti < 8:
                            return vh_t[h][:, ti * 128:(ti + 1) * 128]
                        return v_sb[ti - 8][:, h * 128:(h + 1) * 128]

                    for qc in range(2):
                        ci = h * 2 + qc
                        n_kv = 12 + 4 * qc
                        n_g = n_kv // 2
                        q_ap = qk_t[h][:, qc * 512:(qc + 1) * 512]
                        attn_ps = psAT.tile([128, 512], FP, name="attn", bufs=1)
                        den_ps = psDN.tile([128, 512], FP, name="den", bufs=1)
                        p_acc = pap.tile([128, 1024], BF, name="pacc", bufs=2)
                        p_list = [None] * n_g

                        def emit_da(g, n_kv=n_kv, attn_ps=attn_ps, p_list=p_list,
                                    v_src=v_src):
                            for j in range(2):
                                ti = 2 * g + j
                                nc.tensor.matmul(
                                    attn_ps[:, :], v_src(ti),
                                    p_list[g][:, j * 512:(j + 1) * 512],
                                    start=(ti == 0), stop=(ti == n_kv - 1))

                        for g in range(n_g):
                            sg = psSG.tile([128, 1024], FP, name="sg", bufs=2)
                            for j in range(2):
                                nc.tensor.matmul(
                                    sg[:, j * 512:(j + 1) * 512],
                                    k_src(2 * g + j), q_ap,
                                    start=True, stop=True)
                            p = pp.tile([128, 1024], BF, name="p", bufs=4)
                            di = g - (n_g - 2)
                            if di >= 0:
                                s_sb = sxp.tile([128, 1024], FP, name="sx", bufs=2)
                                nc.vector.tensor_tensor(
                                    out=s_sb[:, :], in0=sg[:, :],
                                    in1=mask_t[di][:, :], op=AX.add)
                                nc.scalar.activation(p[:, :], s_sb[:, :], AF.Exp,
                                                     scale=INV_NORM)
                            else:
                                nc.scalar.activation(p[:, :], sg[:, :], AF.Exp,
                                                     scale=INV_NORM)
                            p_list[g] = p
                            if g == 0:
                                nc.vector.tensor_copy(p_acc[:, :], p[:, :])
                            else:
                                nc.vector.tensor_tensor(
                                    out=p_acc[:, :], in0=p_acc[:, :], in1=p[:, :],
                                    op=AX.add)
                            if g >= LAG:
                                emit_da(g - LAG)
                        # dense filler for previous b (PE keeps busy while the
                        # exp chain of the last groups completes)
                        if at_prev is not None:
                            emit_dense(range(ci * 4, ci * 4 + 4), at_prev,
                                       c0_prev, psC)
                        for g in range(max(n_g - LAG, 0), n_g):
                            emit_da(g)
                        for j in range(2):
                            nc.tensor.matmul(
                                den_ps[:, :], ones128[:, :],
                                p_acc[:, j * 512:(j + 1) * 512],
                                start=(j == 0), stop=(j == 1))
                        rec = recp.tile([128, 512], FP, name="rec", bufs=2)
                        nc.vector.reciprocal_approx_fast(rec[:, :], den_ps[:, :])
                        at = atp.tile([128, 512], BF, name=f"at{h}_{qc}", bufs=2)
                        nc.vector.tensor_tensor(
                            out=at[:, :], in0=attn_ps[:, :], in1=rec[:, :],
                            op=AX.mult)
                        at_cur[h][qc] = at

            at_prev = at_cur
            c0_prev = c0

        # ============ tail: dense for b = B-1 ============
        with ExitStack() as cctx:
            psC2 = cctx.enter_context(
                tc.tile_pool(name="psCtail", bufs=1, space="PSUM"))
            emit_dense(range(32), at_prev, c0_prev, psC2)

    nc.compile()
    return nc


_NC = None
_LAST_EXEC_NS = None


def _host_prep(hidden_states, w_qkv, w_dense, past_key, past_value,
               block_offsets, position_ids_1d):
    xT = np.ascontiguousarray(np.asarray(hidden_states, np.float32)[0].T).astype(BF_NP)
    w_qkv = np.asarray(w_qkv, np.float32)
    w_dense = np.asarray(w_dense, np.float32)
    bo = np.asarray(block_offsets)
    pos = np.asarray(position_ids_1d)

    inv_freq = (1.0 / (ROPE_BASE ** (np.arange(0, D, 2, dtype=np.float32) / D))).astype(np.float32)
    f2 = np.concatenate([inv_freq, inv_freq]).astype(np.float32)
    ang = pos.astype(np.float32)[None, :] * f2[:, None]          # [128, T]
    cosT = np.cos(ang).astype(np.float32)
    sinT = np.sin(ang).astype(np.float32)

    i = np.arange(128)[:, None]
    j = np.arange(512)[None, :]
    m4 = [np.where(i + 128 * d <= j, np.float32(0.0), np.float32(NEG)) for d in range(4)]
    masksAB = np.stack([
        np.concatenate([m4[0], m4[1]], axis=1),
        np.concatenate([m4[2], m4[3]], axis=1),
    ]).astype(np.float32)                                        # [2, 128, 1024]

    nhb = HIST // 64                                             # history blocks per seq
    hist_k = np.asarray(past_key)[bo[:, :nhb]].reshape(B, HIST, NH, D)
    hist_v = np.asarray(past_value)[bo[:, :nhb]].reshape(B, HIST, NH, D)
    # [B, NH, D, HIST] (d-major keys)
    kTh_all = hist_k.transpose(0, 2, 3, 1).astype(BF_NP)
    # [B, NH, 128, 8*128]: vhp[b,h,p,c*128+d] = hist_v[b, c*128+p, h, d]
    vhp_all = hist_v.reshape(B, 8, 128, NH, D).transpose(0, 3, 2, 1, 4) \
        .reshape(B, NH, 128, HIST).astype(BF_NP)

    wq = w_qkv.reshape(H, NH, 3, D)
    in_maps = []
    for c in range(NCORES):
        hs = slice(c * HPC, (c + 1) * HPC)
        wqk2 = np.stack([
            np.ascontiguousarray(wq[:, hs, 0, :].reshape(H, HPC * D)),
            np.ascontiguousarray(wq[:, hs, 1, :].reshape(H, HPC * D)),
        ]).astype(BF_NP)
        in_maps.append({
            "xT": xT,
            "wqk2": wqk2,
            "w_v": np.ascontiguousarray(wq[:, hs, 2, :].reshape(H, HPC * D)).astype(BF_NP),
            "wd": np.ascontiguousarray(w_dense[c * HPC * D:(c + 1) * HPC * D, :]).astype(BF_NP),
            "kTh": np.ascontiguousarray(kTh_all[:, hs]),
            "vhp": np.ascontiguousarray(vhp_all[:, hs]),
            "cosT": cosT,
            "sinT": sinT,
            "masksAB": masksAB,
        })
    return in_maps


def kernel(hidden_states, w_qkv, w_dense, past_key, past_value,
           block_offsets, position_ids_1d):
    global _NC, _LAST_EXEC_NS
    if _NC is None:
        _NC = _build()
    in_maps = _host_prep(hidden_states, w_qkv, w_dense, past_key, past_value,
                         block_offsets, position_ids_1d)
    res = bass_utils.run_bass_kernel_spmd(_NC, in_maps, core_ids=list(range(NCORES)))
    _LAST_EXEC_NS = getattr(res, "exec_time_ns", None)
    acc = np.zeros((H, T), np.float32)
    for c in range(NCORES):
        acc += np.asarray(res.results[c]["outT"], dtype=np.float32)
    return np.ascontiguousarray(acc.T).reshape(1, T, H).astype(np.float32)


# revision 6
# speedup vs baseline: 1.6373x; 1.6373x over previous
import numpy as np
import ml_dtypes
import concourse.bass as bass
import concourse.bacc as bacc
import concourse.tile as tile
import concourse.mybir as mybir
from concourse import bass_utils
from contextlib import ExitStack

B = 4
QL = 1024
HIST = 1024
KVL = 2048
H = 4096
NH = 32
D = 128
T = 4096
NCORES = 8
HPC = NH // NCORES          # 4 heads per core
ROPE_BASE = 10000.0
INV_NORM = 1.0 / float(np.sqrt(D))
NEG = -1.0e30

FP = mybir.dt.float32
BF = mybir.dt.bfloat16
F16 = mybir.dt.float16
AX = mybir.AluOpType
AF = mybir.ActivationFunctionType
BF_NP = ml_dtypes.bfloat16
F16_NP = np.float16

LAG = 3   # attention PV software-pipeline depth (groups)


def _build():
    nc = bacc.Bacc("TRN2", num_devices=NCORES)
    xT = nc.dram_tensor("xT", [H, T], BF, kind="ExternalInput")
    # packed qk weights: [sweep, kk, 128, half*512 + m4*128 + d]
    wqk2 = nc.dram_tensor("wqk2", [2, 16, 128, 1024], BF, kind="ExternalInput")
    w_v = nc.dram_tensor("w_v", [H, HPC * D], BF, kind="ExternalInput")
    wd = nc.dram_tensor("wd", [HPC * D, H], BF, kind="ExternalInput")
    kTh = nc.dram_tensor("kTh", [B, HPC, D, HIST], BF, kind="ExternalInput")
    vhp = nc.dram_tensor("vhp", [B, HPC, 128, HIST], BF, kind="ExternalInput")
    cosT = nc.dram_tensor("cosT", [D, T], F16, kind="ExternalInput")
    sinT = nc.dram_tensor("sinT", [D, T], F16, kind="ExternalInput")
    masksAB = nc.dram_tensor("masksAB", [2, D, 1024], BF, kind="ExternalInput")
    outT = nc.dram_tensor("outT", [H, T], F16, kind="ExternalOutput")

    with tile.TileContext(nc) as tc, ExitStack() as top:
        # ---- constants ----
        cpool = top.enter_context(tc.tile_pool(name="const", bufs=1))
        ones_f = cpool.tile([128, 128], FP)
        nc.vector.memset(ones_f[:, :], 1.0)
        ones128 = cpool.tile([128, 128], BF)
        nc.scalar.copy(ones128[:, :], ones_f[:, :])

        # ---- resident weights / masks ----
        wvp = top.enter_context(tc.tile_pool(name="wvp", bufs=1))
        wdp = top.enter_context(tc.tile_pool(name="wdp", bufs=1))
        mkp = top.enter_context(tc.tile_pool(name="mkp", bufs=1))
        wv_sb = [wvp.tile([128, HPC * D], BF, name=f"wv{k}") for k in range(32)]
        wd_sb = [wdp.tile([128, H], BF, name=f"wd{k}") for k in range(HPC)]
        mask_t = [mkp.tile([128, 1024], BF, name=f"mask{d}") for d in range(2)]

        # ---- rotating pools ----
        xp = top.enter_context(tc.tile_pool(name="xp", bufs=1))
        wqp = top.enter_context(tc.tile_pool(name="wqp", bufs=1))
        qkp = top.enter_context(tc.tile_pool(name="qkp", bufs=1))
        vsp = top.enter_context(tc.tile_pool(name="vsp", bufs=1))
        atp = top.enter_context(tc.tile_pool(name="atp", bufs=1))
        csp = top.enter_context(tc.tile_pool(name="csp", bufs=1))
        khp = top.enter_context(tc.tile_pool(name="khp", bufs=1))
        vhp_p = top.enter_context(tc.tile_pool(name="vhp", bufs=1))
        rotp = top.enter_context(tc.tile_pool(name="rotp", bufs=1))
        pp = top.enter_context(tc.tile_pool(name="pp", bufs=1))
        pap = top.enter_context(tc.tile_pool(name="pap", bufs=1))
        recp = top.enter_context(tc.tile_pool(name="recp", bufs=1))
        op_ = top.enter_context(tc.tile_pool(name="op", bufs=1))

        at_prev = None
        c0_prev = 0
        x_t = [None] * 32
        cs_t = [None, None]

        def load_x(b, lo, hi):
            c0 = b * QL
            for k in range(lo, hi):
                xt = xp.tile([128, 1024], BF, name=f"x{k}", bufs=1)
                nc.sync.dma_start(
                    out=xt[:, :], in_=xT[k * 128:(k + 1) * 128, c0:c0 + QL])
                x_t[k] = xt
            if hi == 32:
                ct = csp.tile([128, 1024], F16, name="cos", bufs=1)
                nc.sync.dma_start(out=ct[:, :], in_=cosT[:, c0:c0 + QL])
                st = csp.tile([128, 1024], F16, name="sin", bufs=1)
                nc.sync.dma_start(out=st[:, :], in_=sinT[:, c0:c0 + QL])
                cs_t[0], cs_t[1] = ct, st

        def emit_dense(ms, at_src, c0_dst, psC, drains):
            for m in ms:
                pso = psC.tile([128, 1024], FP, name="pso", bufs=drains)
                for qc2 in range(2):
                    for k4 in range(HPC):
                        nc.tensor.matmul(
                            pso[:, qc2 * 512:(qc2 + 1) * 512],
                            wd_sb[k4][:, m * 128:(m + 1) * 128],
                            at_src[k4][qc2][:, :],
                            start=(k4 == 0), stop=(k4 == HPC - 1))
                o = op_.tile([128, 1024], F16, name="o", bufs=2)
                if m % 2 == 0:
                    nc.scalar.copy(o[:, :], pso[:, :])
                else:
                    nc.vector.tensor_copy(o[:, :], pso[:, :])
                nc.sync.dma_start(
                    out=outT[m * 128:(m + 1) * 128, c0_dst:c0_dst + QL],
                    in_=o[:, :])

        for b in range(B):
            c0 = b * QL
            qk_t = [None] * 8
            v_sb = []

            # ============ stage A1: q^T,k^T = w^T x with RoPE ============
            with ExitStack() as actx:
                psA = actx.enter_context(
                    tc.tile_pool(name=f"psA{b}", bufs=1, space="PSUM"))
                for sweep in range(2):
                    psa = [psA.tile([128, 1024], FP, name=f"psa{m}", bufs=1)
                           for m in range(4)]
                    for kk in range(16):
                        if b == 0 and sweep == 0:
                            load_x(0, 2 * kk, 2 * kk + 2)
                            if kk == 15:
                                load_x(0, 32, 32)  # cos/sin only
                        wq = wqp.tile([128, 1024], BF, name="wq", bufs=3)
                        nc.sync.dma_start(out=wq[:, :], in_=wqk2[sweep, kk, :, :])
                        for half in range(2):
                            xt = x_t[2 * kk + half]
                            for m4 in range(4):
                                for ns in range(2):
                                    nc.tensor.matmul(
                                        psa[m4][:, ns * 512:(ns + 1) * 512],
                                        wq[:, half * 512 + m4 * 128:
                                           half * 512 + (m4 + 1) * 128],
                                        xt[:, ns * 512:(ns + 1) * 512],
                                        start=(kk == 0 and half == 0),
                                        stop=(kk == 15 and half == 1))
                    if b == 0 and sweep == 0:
                        for k in range(32):
                            nc.sync.dma_start(
                                out=wv_sb[k][:, :],
                                in_=w_v[k * 128:(k + 1) * 128, :])
                    if b == 0 and sweep == 1:
                        for k in range(HPC):
                            nc.sync.dma_start(
                                out=wd_sb[k][:, :], in_=wd[k * 128:(k + 1) * 128, :])
                        for d in range(2):
                            nc.sync.dma_start(
                                out=mask_t[d][:, :], in_=masksAB[d, :, :])
                    # rope drain: qk = psa*cos + rotate_half(psa)*sin
                    for m4 in range(4):
                        rot = rotp.tile([128, 1024], F16, name="rot", bufs=1)
                        nc.scalar.mul(rot[0:64, :], psa[m4][64:128, :], -1.0)
                        nc.scalar.copy(rot[64:128, :], psa[m4][0:64, :])
                        qk = qkp.tile([128, 1024], BF, name=f"qk{sweep * 4 + m4}")
                        nc.vector.tensor_tensor(
                            out=qk[:, :], in0=psa[m4][:, :], in1=cs_t[0][:, :],
                            op=AX.mult)
                        nc.vector.tensor_tensor(
                            out=rot[:, :], in0=rot[:, :], in1=cs_t[1][:, :],
                            op=AX.mult)
                        nc.vector.tensor_tensor(
                            out=qk[:, :], in0=qk[:, :], in1=rot[:, :], op=AX.add)
                        qk_t[sweep * 4 + m4] = qk

            # ============ stage A2: new V (token-major) ============
            with ExitStack() as a2ctx:
                psV = a2ctx.enter_context(
                    tc.tile_pool(name=f"psV{b}", bufs=1, space="PSUM"))
                psv = [psV.tile([128, 512], FP, name=f"psv{t}", bufs=1)
                       for t in range(8)]
                for k in range(32):
                    xt = x_t[k]
                    for t in range(8):
                        nc.tensor.matmul(
                            psv[t][:, :], xt[:, t * 128:(t + 1) * 128],
                            wv_sb[k][:, :], start=(k == 0), stop=(k == 31))
                kh_t = [khp.tile([128, HIST], BF, name="kh", bufs=2)]
                nc.sync.dma_start(out=kh_t[0][:, :], in_=kTh[b, 0, :, :])
                vh_t = [vhp_p.tile([128, HIST], BF, name="vh", bufs=2)]
                nc.sync.dma_start(out=vh_t[0][:, :], in_=vhp[b, 0, :, :])
                for t in range(8):
                    vt = vsp.tile([128, 512], BF, name=f"v{t}")
                    if t % 2 == 0:
                        nc.scalar.copy(vt[:, :], psv[t][:, :])
                    else:
                        nc.vector.tensor_copy(vt[:, :], psv[t][:, :])
                    v_sb.append(vt)

            # ============ stage B(b) attention ⊗ C(b-1) dense ============
            at_cur = [[None, None] for _ in range(HPC)]
            with ExitStack() as bctx:
                psSG = bctx.enter_context(
                    tc.tile_pool(name=f"psSG{b}", bufs=1, space="PSUM"))
                psAT = bctx.enter_context(
                    tc.tile_pool(name=f"psAT{b}", bufs=1, space="PSUM"))
                psDN = bctx.enter_context(
                    tc.tile_pool(name=f"psDN{b}", bufs=1, space="PSUM"))
                psC = bctx.enter_context(
                    tc.tile_pool(name=f"psC{b}", bufs=1, space="PSUM"))

                for h in range(HPC):
                    kh_cur, vh_cur = kh_t[0], vh_t[0]

                    def k_src(ti, h=h, kh_cur=kh_cur):
                        if ti < 8:
                            return kh_cur[:, ti * 128:(ti + 1) * 128]
                        return qk_t[4 + h][:, (ti - 8) * 128:(ti - 7) * 128]

                    def v_src(ti, h=h, vh_cur=vh_cur):
                        if ti < 8:
                            return vh_cur[:, ti * 128:(ti + 1) * 128]
                        return v_sb[ti - 8][:, h * 128:(h + 1) * 128]

                    for qc in range(2):
                        ci = h * 2 + qc
                        n_kv = 12 + 4 * qc
                        n_g = n_kv // 2
                        q_ap = qk_t[h][:, qc * 512:(qc + 1) * 512]
                        attn_ps = psAT.tile([128, 512], FP, name="attn", bufs=1)
                        den_ps = psDN.tile([128, 512], FP, name="den", bufs=1)
                        p_acc = pap.tile([128, 1024], BF, name="pacc", bufs=1)
                        p_list = [None] * n_g

                        def emit_da(gi, n_kv=n_kv, attn_ps=attn_ps, p_list=p_list,
                                    v_src=v_src):
                            for j in range(2):
                                ti = 2 * gi + j
                                nc.tensor.matmul(
                                    attn_ps[:, :], v_src(ti),
                                    p_list[gi][:, j * 512:(j + 1) * 512],
                                    start=(ti == 0), stop=(ti == n_kv - 1))

                        for gi in range(n_g):
                            sg = psSG.tile([128, 1024], FP, name="sg", bufs=2)
                            for j in range(2):
                                nc.tensor.matmul(
                                    sg[:, j * 512:(j + 1) * 512],
                                    k_src(2 * gi + j), q_ap,
                                    start=True, stop=True)
                            p = pp.tile([128, 1024], BF, name="p", bufs=3)
                            di = gi - (n_g - 2)
                            if di >= 0:
                                nc.vector.tensor_tensor(
                                    out=sg[:, :], in0=sg[:, :],
                                    in1=mask_t[di][:, :], op=AX.add)
                            nc.scalar.activation(p[:, :], sg[:, :], AF.Exp,
                                                 scale=INV_NORM)
                            p_list[gi] = p
                            if gi == 0:
                                nc.vector.tensor_copy(p_acc[:, :], p[:, :])
                            else:
                                nc.vector.tensor_tensor(
                                    out=p_acc[:, :], in0=p_acc[:, :], in1=p[:, :],
                                    op=AX.add)
                            if gi >= LAG:
                                emit_da(gi - LAG)
                        # issue rolling prefetches early in the chunk stream
                        if qc == 0 and h + 1 < HPC:
                            kh_n = khp.tile([128, HIST], BF, name="kh", bufs=2)
                            nc.sync.dma_start(out=kh_n[:, :],
                                              in_=kTh[b, h + 1, :, :])
                            vh_n = vhp_p.tile([128, HIST], BF, name="vh", bufs=2)
                            nc.sync.dma_start(out=vh_n[:, :],
                                              in_=vhp[b, h + 1, :, :])
                            kh_t[0], vh_t[0] = kh_n, vh_n
                        if b + 1 < B:
                            if ci == 1:
                                load_x(b + 1, 0, 16)
                            elif ci == 3:
                                load_x(b + 1, 16, 32)
                        # dense filler for previous b
                        if at_prev is not None:
                            emit_dense(range(ci * 4, ci * 4 + 4), at_prev,
                                       c0_prev, psC, 1)
                        for gi in range(max(n_g - LAG, 0), n_g):
                            emit_da(gi)
                        for j in range(2):
                            nc.tensor.matmul(
                                den_ps[:, :], ones128[:, :],
                                p_acc[:, j * 512:(j + 1) * 512],
                                start=(j == 0), stop=(j == 1))
                        rec = recp.tile([128, 512], FP, name="rec", bufs=1)
                        nc.vector.reciprocal_approx_fast(rec[:, :], den_ps[:, :])
                        at = atp.tile([128, 512], BF, name=f"at{h}_{qc}", bufs=2)
                        nc.vector.tensor_tensor(
                            out=at[:, :], in0=attn_ps[:, :], in1=rec[:, :],
                            op=AX.mult)
                        at_cur[h][qc] = at

            at_prev = at_cur
            c0_prev = c0

        # ============ tail: dense for b = B-1 ============
        with ExitStack() as cctx:
            psC2 = cctx.enter_context(
                tc.tile_pool(name="psCtail", bufs=1, space="PSUM"))
            emit_dense(range(32), at_prev, c0_prev, psC2, 3)

    nc.compile()
    return nc


_NC = None
_LAST_EXEC_NS = None


def _host_prep(hidden_states, w_qkv, w_dense, past_key, past_value,
               block_offsets, position_ids_1d):
    xT = np.ascontiguousarray(np.asarray(hidden_states, np.float32)[0].T).astype(BF_NP)
    w_qkv = np.asarray(w_qkv, np.float32)
    w_dense = np.asarray(w_dense, np.float32)
    bo = np.asarray(block_offsets)
    pos = np.asarray(position_ids_1d)

    inv_freq = (1.0 / (ROPE_BASE ** (np.arange(0, D, 2, dtype=np.float32) / D))).astype(np.float32)
    f2 = np.concatenate([inv_freq, inv_freq]).astype(np.float32)
    ang = pos.astype(np.float32)[None, :] * f2[:, None]          # [128, T]
    cosT = np.cos(ang).astype(F16_NP)
    sinT = np.sin(ang).astype(F16_NP)

    i = np.arange(128)[:, None]
    j = np.arange(512)[None, :]
    m4 = [np.where(i + 128 * d <= j, np.float32(0.0), np.float32(NEG)) for d in range(4)]
    masksAB = np.stack([
        np.concatenate([m4[0], m4[1]], axis=1),
        np.concatenate([m4[2], m4[3]], axis=1),
    ]).astype(BF_NP)                                             # [2, 128, 1024]

    nhb = HIST // 64                                             # history blocks per seq
    hist_k = np.asarray(past_key)[bo[:, :nhb]].reshape(B, HIST, NH, D)
    hist_v = np.asarray(past_value)[bo[:, :nhb]].reshape(B, HIST, NH, D)
    # [B, NH, D, HIST] (d-major keys)
    kTh_all = hist_k.transpose(0, 2, 3, 1).astype(BF_NP)
    # [B, NH, 128, 8*128]: vhp[b,h,p,c*128+d] = hist_v[b, c*128+p, h, d]
    vhp_all = hist_v.reshape(B, 8, 128, NH, D).transpose(0, 3, 2, 1, 4) \
        .reshape(B, NH, 128, HIST).astype(BF_NP)

    wq = w_qkv.reshape(H, NH, 3, D)
    in_maps = []
    for c in range(NCORES):
        hs = slice(c * HPC, (c + 1) * HPC)
        # packed [2, 16, 128, 1024]: [s, kk, p, half*512+col]
        wqk2 = np.stack([
            wq[:, hs, s, :].reshape(16, 2, 128, 512).transpose(0, 2, 1, 3)
            .reshape(16, 128, 1024)
            for s in range(2)
        ]).astype(BF_NP)
        in_maps.append({
            "xT": xT,
            "wqk2": np.ascontiguousarray(wqk2),
            "w_v": np.ascontiguousarray(wq[:, hs, 2, :].reshape(H, HPC * D)).astype(BF_NP),
            "wd": np.ascontiguousarray(w_dense[c * HPC * D:(c + 1) * HPC * D, :]).astype(BF_NP),
            "kTh": np.ascontiguousarray(kTh_all[:, hs]),
            "vhp": np.ascontiguousarray(vhp_all[:, hs]),
            "cosT": cosT,
            "sinT": sinT,
            "masksAB": masksAB,
        })
    return in_maps


def kernel(hidden_states, w_qkv, w_dense, past_key, past_value,
           block_offsets, position_ids_1d):
    global _NC, _LAST_EXEC_NS
    if _NC is None:
        _NC = _build()
    in_maps = _host_prep(hidden_states, w_qkv, w_dense, past_key, past_value,
                         block_offsets, position_ids_1d)
    res = bass_utils.run_bass_kernel_spmd(_NC, in_maps, core_ids=list(range(NCORES)))
    _LAST_EXEC_NS = getattr(res, "exec_time_ns", None)
    acc = np.zeros((H, T), np.float32)
    for c in range(NCORES):
        acc += np.asarray(res.results[c]["outT"], dtype=np.float32)
    return np.ascontiguousarray(acc.T).reshape(1, T, H).astype(np.float32)
